# revision 11
# baseline (speedup 1.0000x reference)
"""Trainium2 Bass kernel for nn_DeepLSTMDecoderLayer.

Layer: c = srcMHA(x, memory) + tgtMHA(x, x causal); hiddens = LSTM([x, c]);
out = FFN(hiddens).   Shapes: B=16, T=S=512, H=1024, NH=16, F=4096.

Sharding: data-parallel over batch across 8 cores (B_local=2), weights
replicated.  All matmuls bf16 operands / f32 accumulate.

Layouts are feature-major ("transposed") throughout:
  - x^T / mem^T [1024 emb, 1024 tok] bf16, tok = b*512 + t
  - projections produce Q^T/K^T [emb-d, tok] and V token-major [tok, d]
  - softmax with s on partitions: exp(logits^T - 8) (constant shift cancels
    in normalization), denominator via ones-vector matmul, normalization
    applied after P@V via a rank-1 PE broadcast of 1/denom (hi+lo bf16
    split keeps it ~f32-exact)
  - LSTM: the input-part gate pre-activations G = [x,c] @ W_x + b are
    precomputed to DRAM with gate columns permuted to [j|i|f|o] and stored
    p-major (m = p*32 + c) so each step's slice is one clean [128, 32] DMA;
    the recurrence runs 512 sequential W_h-stationary steps (columns
    c-major, m = c*128 + p) accumulating into 4 per-gate-type PSUM tiles
    [128, q, b]; h-history lives in SBUF [128, q, b, slot] bf16 and feeds
    the FFN directly.
"""
import sys

if '/opt/trn_rl_repo' not in sys.path:
    sys.path.insert(0, '/opt/trn_rl_repo')

import numpy as np
import ml_dtypes
from contextlib import ExitStack

import concourse.bass as bass
import concourse.mybir as mybir
import concourse.tile as tile
import bass_rust
from concourse.bass import ds
from concourse.bass_utils import run_bass_kernel_spmd

F32 = mybir.dt.float32
BF16 = mybir.dt.bfloat16
AF = mybir.ActivationFunctionType
BFnp = ml_dtypes.bfloat16

B, T, S, H, NH, F = 16, 512, 512, 1024, 16, 4096
HD = H // NH
NCORES = 8
BL = B // NCORES          # 2 batch items per core
TOK = BL * T              # 1024 tokens per core
SHIFT = 8.0               # constant softmax shift (cancels in normalization)

# gate-type order [j, i, f, o]: tanh block first, then contiguous sigmoids.
_JIFO = np.concatenate([np.arange(1024, 2048), np.arange(0, 1024),
                        np.arange(2048, 3072), np.arange(3072, 4096)])


# ----------------------------------------------------------------------------
# walrus workaround: this neuronx-cc rejects instructions with >1 sync wait
# ----------------------------------------------------------------------------
_wsplit_ctr = [0]


def _split_excess_waits(nc, max_waits=1):
    n = 0
    for f in nc.m.functions:
        for blk in f.blocks:
            out = []
            changed = False
            for inst in blk.instructions:
                si = inst.sync_info
                if si is not None and len(si.on_wait) > max_waits:
                    waits = list(si.on_wait)
                    excess = waits[max_waits:]
                    for i in range(0, len(excess), max_waits):
                        _wsplit_ctr[0] += 1
                        out.append(mybir.InstNoOp(
                            name=f"wsplit-{_wsplit_ctr[0]}",
                            engine=inst.engine,
                            sync_info=bass_rust.SyncInfo(
                                on_wait=excess[i:i + max_waits], on_update=[]),
                            bass_nofuse=True))
                    inst.sync_info = bass_rust.SyncInfo(
                        on_wait=waits[:max_waits], on_update=list(si.on_update))
                    changed = True
                    n += 1
                out.append(inst)
            if changed:
                blk.instructions = out
    return n


# ----------------------------------------------------------------------------
# device program
# ----------------------------------------------------------------------------
def _build_program(tsteps=T):
    nc = bass.Bass()
    dp = lambda name, shape, dt=BF16: nc.declare_dram_parameter(
        name, list(shape), dt, isOutput=False)

    xT_e = dp("xT", [8, 128, TOK])
    memT_e = dp("memT", [8, 128, TOK])
    wq_e = {a: dp(f"{a}_wq", [H, H]) for a in ("src", "tgt")}
    wk_e = {a: dp(f"{a}_wk", [H, H]) for a in ("src", "tgt")}
    wv_e = {a: dp(f"{a}_wv", [H, H]) for a in ("src", "tgt")}
    wo_e = {a: dp(f"{a}_wo", [H, H]) for a in ("src", "tgt")}
    bq_e = {a: dp(f"{a}_bq", [128, 8], F32) for a in ("src", "tgt")}
    bk_e = {a: dp(f"{a}_bk", [128, 8], F32) for a in ("src", "tgt")}
    bv_e = {a: dp(f"{a}_bv", [1, H]) for a in ("src", "tgt")}
    bo_e = {a: dp(f"{a}_bo", [128, 8], F32) for a in ("src", "tgt")}
    srcb_e = dp("src_bshift", [128, BL * 4], F32)     # src_bias - SHIFT
    tgtb_e = dp("tgt_bT", [4, 128, T])                # bias^T [s, q] bf16
    wx_e = dp("Wx", [16, 128, 4096])
    brow_e = dp("b_row", [1, 4096])
    wh_e = dp("Wh", [8, 128, 4096])
    w1_e = dp("W1", [8, 128, F])
    b1_e = dp("b1T", [128, 32], F32)
    w2_e = dp("W2", [32, 128, H])
    b2_e = dp("b2T", [128, 8], F32)

    otok = BL * tsteps
    yT_e = nc.declare_dram_parameter("yT", [8, 128, otok], F32, isOutput=True)
    G_d = nc.dram_tensor("Gd", [TOK, 128, 32], F32)

    with tile.TileContext(nc) as tc:
        with ExitStack() as top:
            persist = top.enter_context(tc.tile_pool(name="persist", bufs=1))
            smalls = top.enter_context(tc.tile_pool(name="smalls", bufs=1))

            xT = [persist.tile([128, TOK], BF16, tag=f"xT{k}", name=f"xT{k}") for k in range(8)]
            cT = [[persist.tile([128, T], BF16, tag=f"cT{b}_{m}", name=f"cT{b}_{m}")
                   for m in range(8)] for b in range(BL)]
            for k in range(8):
                nc.sync.dma_start(out=xT[k], in_=xT_e[k])

            ones_col = smalls.tile([1, 128], BF16, tag="ones_col", name="ones_col")
            nc.vector.memset(ones_col, 1.0)
            ones_128 = smalls.tile([128, 1], BF16, tag="ones_128", name="ones_128")
            nc.vector.memset(ones_128, 1.0)
            ones_1x64 = smalls.tile([1, 64], BF16, tag="ones_1x64", name="ones_1x64")
            nc.vector.memset(ones_1x64, 1.0)
            srcb_t = smalls.tile([128, BL * 4], F32, tag="srcb", name="srcb")
            nc.sync.dma_start(out=srcb_t, in_=srcb_e[:, :])
            negshift = smalls.tile([128, 1], F32, tag="negshift", name="negshift")
            nc.vector.memset(negshift, -SHIFT)

            # ---------------- phase A: both attentions -> cT ----------------
            with tc.tile_pool(name="attc", bufs=1) as attc, \
                 tc.tile_pool(name="wstream", bufs=2) as wpool, \
                 tc.tile_pool(name="qkv", bufs=1) as qkv, \
                 tc.tile_pool(name="epool", bufs=2) as epool, \
                 tc.tile_pool(name="ps_lt", bufs=4, space="PSUM") as ps_lt, \
                 tc.tile_pool(name="ps_s", bufs=1, space="PSUM") as ps_s, \
                 tc.tile_pool(name="ps_o", bufs=1, space="PSUM") as ps_o, \
                 tc.tile_pool(name="ps_rb", bufs=1, space="PSUM") as ps_rb:

                memT = [attc.tile([128, TOK], BF16, tag=f"memT{k}", name=f"memT{k}")
                        for k in range(8)]
                for k in range(8):
                    nc.sync.dma_start(out=memT[k], in_=memT_e[k])
                tgtb = [attc.tile([128, T], BF16, tag=f"tgtb{st}", name=f"tgtb{st}")
                        for st in range(4)]
                for st in range(4):
                    nc.sync.dma_start(out=tgtb[st], in_=tgtb_e[st])

                for attn in ("src", "tgt"):
                    kvT = memT if attn == "src" else xT

                    def load_w(ext):
                        w_sb = [wpool.tile([128, H], BF16, tag=f"w{k}", name=f"w{k}")
                                for k in range(8)]
                        for k in range(8):
                            nc.sync.dma_start(out=w_sb[k],
                                              in_=ext[ds(k * 128, 128), :])
                        return w_sb

                    bq_t = smalls.tile([128, 8], F32, tag=f"bq_{attn}", name=f"bq_{attn}")
                    nc.sync.dma_start(out=bq_t, in_=bq_e[attn][:, :])
                    bk_t = smalls.tile([128, 8], F32, tag=f"bk_{attn}", name=f"bk_{attn}")
                    nc.sync.dma_start(out=bk_t, in_=bk_e[attn][:, :])
                    bv_t = smalls.tile([1, H], BF16, tag=f"bv_{attn}", name=f"bv_{attn}")
                    nc.sync.dma_start(out=bv_t, in_=bv_e[attn][:, :])

                    # --- projections: Q^T, K^T (emb-major), V (token-major)
                    QT, KT, VV = [], [], []
                    w_sb = load_w(wq_e[attn])
                    for m in range(8):
                        q_sb = qkv.tile([128, TOK], BF16, tag=f"qT{m}", name=f"qT{m}")
                        QT.append(q_sb)
                        for nch in range(2):
                            ps = ps_lt.tile([128, 512], F32, tag="lt", name="lt")
                            for k in range(8):
                                nc.tensor.matmul(
                                    ps, w_sb[k][:, ds(m * 128, 128)],
                                    xT[k][:, ds(nch * 512, 512)],
                                    start=(k == 0), stop=(k == 7))
                            nc.scalar.activation(
                                out=q_sb[:, ds(nch * 512, 512)], in_=ps,
                                func=AF.Identity, bias=bq_t[:, ds(m, 1)], scale=1.0)
                    w_sb = load_w(wk_e[attn])
                    for m in range(8):
                        k_sb = qkv.tile([128, TOK], BF16, tag=f"kT{m}", name=f"kT{m}")
                        KT.append(k_sb)
                        for nch in range(2):
                            ps = ps_lt.tile([128, 512], F32, tag="lt", name="lt")
                            for k in range(8):
                                nc.tensor.matmul(
                                    ps, w_sb[k][:, ds(m * 128, 128)],
                                    kvT[k][:, ds(nch * 512, 512)],
                                    start=(k == 0), stop=(k == 7))
                            nc.scalar.activation(
                                out=k_sb[:, ds(nch * 512, 512)], in_=ps,
                                func=AF.Identity, bias=bk_t[:, ds(m, 1)], scale=1.0)
                    w_sb = load_w(wv_e[attn])
                    for st in range(8):   # global token tile (b*4 + s_tile)
                        v_sb = qkv.tile([128, H], BF16, tag=f"v{st}", name=f"v{st}")
                        VV.append(v_sb)
                        for nch in range(2):
                            ps = ps_lt.tile([128, 512], F32, tag="lt", name="lt")
                            for k in range(8):
                                nc.tensor.matmul(
                                    ps, kvT[k][:, ds(st * 128, 128)],
                                    w_sb[k][:, ds(nch * 512, 512)],
                                    start=(k == 0), stop=False)
                            nc.tensor.matmul(
                                ps, ones_col, bv_t[:, ds(nch * 512, 512)],
                                start=False, stop=True)
                            nc.vector.tensor_copy(
                                out=v_sb[:, ds(nch * 512, 512)], in_=ps)

                    # --- attention core, concat heads into oT ---
                    oT = [[qkv.tile([128, T], BF16, tag=f"oT{b}_{mm}", name=f"oT{b}_{mm}")
                           for mm in range(8)] for b in range(BL)]
                    for b in range(BL):
                        for h in range(NH):
                            mh, off = h // 2, 64 * (h % 2)
                            lts = []
                            for st in range(4):
                                ps = ps_lt.tile([128, 512], F32, tag="lt", name="lt")
                                lts.append(ps)
                                nc.tensor.matmul(
                                    ps,
                                    KT[mh][ds(off, 64),
                                           ds(b * 512 + st * 128, 128)],
                                    QT[mh][ds(off, 64), ds(b * 512, 512)],
                                    start=True, stop=True)
                            es = []
                            for st in range(4):
                                e_sb = epool.tile([128, 512], BF16,
                                                  tag=f"e{st}", name=f"e{st}")
                                es.append(e_sb)
                                if attn == "tgt":
                                    nc.vector.tensor_add(lts[st], lts[st],
                                                         tgtb[st])
                                    nc.scalar.activation(
                                        out=e_sb, in_=lts[st], func=AF.Exp,
                                        bias=negshift[:, :], scale=1.0)
                                else:
                                    nc.scalar.activation(
                                        out=e_sb, in_=lts[st], func=AF.Exp,
                                        bias=srcb_t[:, ds(b * 4 + st, 1)], scale=1.0)
                            s_ps = ps_s.tile([1, 512], F32, tag="s", name="s")
                            for st in range(4):
                                nc.tensor.matmul(s_ps, ones_128, es[st],
                                                 start=(st == 0),
                                                 stop=(st == 3))
                            r_sb = epool.tile([1, 512], F32, tag="r", name="r")
                            nc.vector.reciprocal(out=r_sb, in_=s_ps)
                            # hi+lo bf16 split of 1/denom (keeps ~f32 accuracy
                            # through the bf16 rank-1 broadcast): two
                            # accumulating K=1 matmuls
                            r_hi = epool.tile([1, 512], BF16, tag="rhi", name="rhi")
                            nc.vector.tensor_copy(out=r_hi, in_=r_sb)
                            r_lo = epool.tile([1, 512], BF16, tag="rlo", name="rlo")
                            nc.vector.tensor_sub(r_lo, r_sb, r_hi)
                            o_ps = ps_o.tile([64, 512], F32, tag="o", name="o")
                            for st in range(4):
                                nc.tensor.matmul(
                                    o_ps,
                                    VV[b * 4 + st][:, ds(h * 64, 64)],
                                    es[st], start=(st == 0), stop=(st == 3))
                            rb_ps = ps_rb.tile([64, 512], F32, tag="rb", name="rb")
                            nc.tensor.matmul(rb_ps, ones_1x64, r_hi,
                                             start=True, stop=False)
                            nc.tensor.matmul(rb_ps, ones_1x64, r_lo,
                                             start=False, stop=True)
                            o_f = epool.tile([64, 512], F32, tag="of", name="of")
                            nc.vector.tensor_copy(out=o_f, in_=o_ps)
                            nc.vector.tensor_mul(
                                oT[b][mh][ds(off, 64), :], o_f, rb_ps)

                    # --- out-projection into cT (src writes, tgt adds) ---
                    bo_t = smalls.tile([128, 8], F32, tag=f"bo_{attn}", name=f"bo_{attn}")
                    nc.sync.dma_start(out=bo_t, in_=bo_e[attn][:, :])
                    w_sb = load_w(wo_e[attn])
                    for b in range(BL):
                        for m in range(8):
                            ps = ps_lt.tile([128, 512], F32, tag="lt", name="lt")
                            for k in range(8):
                                nc.tensor.matmul(
                                    ps, w_sb[k][:, ds(m * 128, 128)],
                                    oT[b][k], start=(k == 0), stop=(k == 7))
                            if attn == "src":
                                nc.scalar.activation(
                                    out=cT[b][m], in_=ps, func=AF.Identity,
                                    bias=bo_t[:, ds(m, 1)], scale=1.0)
                            else:
                                tmp = epool.tile([128, 512], F32, tag="ctmp", name="ctmp")
                                nc.scalar.activation(
                                    out=tmp, in_=ps, func=AF.Identity,
                                    bias=bo_t[:, ds(m, 1)], scale=1.0)
                                nc.vector.tensor_add(cT[b][m], cT[b][m], tmp)

            # ---------------- phase B: G = [x, c] @ Wx + b -> DRAM ----------
            with tc.tile_pool(name="wx", bufs=1) as wxp, \
                 tc.tile_pool(name="gstage", bufs=4) as gst, \
                 tc.tile_pool(name="ps_g", bufs=4, space="PSUM") as ps_g:
                wx_sb = [wxp.tile([128, 4096], BF16, tag=f"wx{k}", name=f"wx{k}")
                         for k in range(16)]
                for k in range(16):
                    nc.sync.dma_start(out=wx_sb[k], in_=wx_e[k])
                brow = smalls.tile([1, 4096], BF16, tag="brow", name="brow")
                nc.sync.dma_start(out=brow, in_=brow_e[:, :])

                for b in range(BL):
                    for tt in range(4):
                        for nch in range(8):
                            ps = ps_g.tile([128, 512], F32, tag="g", name="g")
                            for k in range(8):
                                nc.tensor.matmul(
                                    ps, xT[k][:, ds(b * 512 + tt * 128, 128)],
                                    wx_sb[k][:, ds(nch * 512, 512)],
                                    start=(k == 0), stop=False)
                            for k in range(8):
                                nc.tensor.matmul(
                                    ps, cT[b][k][:, ds(tt * 128, 128)],
                                    wx_sb[8 + k][:, ds(nch * 512, 512)],
                                    start=False, stop=False)
                            nc.tensor.matmul(
                                ps, ones_col, brow[:, ds(nch * 512, 512)],
                                start=False, stop=True)
                            g_sb = gst.tile([128, 512], F32, tag="gsb", name="gsb")
                            nc.vector.tensor_copy(out=g_sb, in_=ps)
                            r0 = b * 512 + tt * 128
                            nc.sync.dma_start(
                                out=G_d[ds(r0, 128), ds(nch * 16, 16), :],
                                in_=g_sb.rearrange("p (a c) -> p a c", a=16))

            # ---------------- phases C+D: recurrence then FFN ---------------
            with tc.tile_pool(name="hstate", bufs=1) as hsp:
                hT = hsp.tile([128, 8, BL, tsteps + 1], BF16, tag="hT", name="hT")
                nc.vector.memset(hT[:, :, :, 0], 0.0)

                with tc.tile_pool(name="wh", bufs=1) as whp, \
                     tc.tile_pool(name="gstep", bufs=1) as gsp, \
                     tc.tile_pool(name="gw", bufs=2) as gwp, \
                     tc.tile_pool(name="ps_r", bufs=2, space="PSUM") as ps_r:
                    wh_sb = [whp.tile([128, 4096], BF16, tag=f"wh{k}", name=f"wh{k}")
                             for k in range(8)]
                    for k in range(8):
                        nc.sync.dma_start(out=wh_sb[k], in_=wh_e[k])
                    c_st = hsp.tile([128, 8, BL], F32, tag="c_st", name="c_st")
                    nc.vector.memset(c_st, 0.0)
                    hstep = hsp.tile([128, 8, BL], BF16, tag="hstep", name="hstep")
                    nc.vector.memset(hstep, 0.0)
                    g_t = gsp.tile([128, 32, BL], F32, tag="g_t", name="g_t")

                    with tc.For_i(0, tsteps, 1) as t:
                        for b in range(BL):
                            nc.sync.dma_start(
                                out=g_t[:, :, b],
                                in_=G_d[ds(t + b * 512, 1), :, :].rearrange(
                                    "one p c -> (one p) c"))
                        psg = [ps_r.tile([128, 8, BL], F32, tag=f"ps{tau}", name=f"ps{tau}")
                               for tau in range(4)]
                        ga = [gwp.tile([128, 8, BL], F32, tag=f"ga{tau}", name=f"ga{tau}")
                              for tau in range(4)]
                        tj = gwp.tile([128, 8, BL], F32, tag="tj", name="tj")
                        si = gwp.tile([128, 8, BL], F32, tag="si", name="si")
                        sf = gwp.tile([128, 8, BL], F32, tag="sf", name="sf")
                        so = gwp.tile([128, 8, BL], F32, tag="so", name="so")
                        t1 = gwp.tile([128, 8, BL], F32, tag="t1", name="t1")
                        m1 = gwp.tile([128, 8, BL], F32, tag="m1", name="m1")
                        tcn = gwp.tile([128, 8, BL], F32, tag="tcn", name="tcn")

                        for tau in range(4):
                            for q in range(8):
                                c = tau * 8 + q
                                for k in range(8):
                                    nc.tensor.matmul(
                                        psg[tau][:, q, :],
                                        wh_sb[k][:, ds(c * 128, 128)],
                                        hstep[:, k, :],
                                        start=(k == 0), stop=(k == 7))
                            nc.vector.tensor_add(
                                ga[tau], psg[tau], g_t[:, ds(tau * 8, 8), :])
                            if tau == 0:
                                nc.scalar.activation(out=tj, in_=ga[0],
                                                     func=AF.Tanh)
                            elif tau == 1:
                                nc.scalar.activation(out=si, in_=ga[1],
                                                     func=AF.Sigmoid)
                                nc.vector.tensor_mul(t1, si, tj)
                            elif tau == 2:
                                nc.scalar.activation(out=sf, in_=ga[2],
                                                     func=AF.Sigmoid)
                                nc.vector.tensor_mul(m1, sf, c_st)
                                nc.vector.tensor_add(c_st, t1, m1)
                                nc.scalar.activation(out=tcn, in_=c_st,
                                                     func=AF.Tanh)
                            else:
                                nc.scalar.activation(out=so, in_=ga[3],
                                                     func=AF.Sigmoid)
                                nc.vector.tensor_mul(hstep, so, tcn)
                                nc.vector.tensor_mul(
                                    hT[:, :, :, ds(t + 1, 1)], so, tcn)

                # ---------------- phase D: FFN --------------------------
                b1_t = smalls.tile([128, 32], F32, tag="b1t", name="b1t")
                nc.sync.dma_start(out=b1_t, in_=b1_e[:, :])
                b2_t = smalls.tile([128, 8], F32, tag="b2t", name="b2t")
                nc.sync.dma_start(out=b2_t, in_=b2_e[:, :])

                with tc.tile_pool(name="f1", bufs=1) as f1p, \
                     tc.tile_pool(name="ps_f", bufs=4, space="PSUM") as ps_f:
                    f1_sb = [f1p.tile([128, BL * tsteps], BF16, tag=f"f1{fc}", name=f"f1{fc}")
                             for fc in range(32)]
                    with tc.tile_pool(name="w1", bufs=1) as w1p:
                        w1_sb = [w1p.tile([128, F], BF16, tag=f"w1{k}", name=f"w1{k}")
                                 for k in range(8)]
                        for k in range(8):
                            nc.sync.dma_start(out=w1_sb[k], in_=w1_e[k])
                        for fc in range(32):
                            for b in range(BL):
                                ps = ps_f.tile([128, tsteps], F32, tag="f", name="f")
                                for k in range(8):
                                    nc.tensor.matmul(
                                        ps, w1_sb[k][:, ds(fc * 128, 128)],
                                        hT[:, k, b, 1:tsteps + 1],
                                        start=(k == 0), stop=(k == 7))
                                nc.scalar.activation(
                                    out=f1_sb[fc][:, ds(b * tsteps, tsteps)],
                                    in_=ps, func=AF.Relu,
                                    bias=b1_t[:, ds(fc, 1)], scale=1.0)

                    with tc.tile_pool(name="w2", bufs=1) as w2p, \
                         tc.tile_pool(name="yout", bufs=4) as yp:
                        w2_sb = [w2p.tile([128, H], BF16, tag=f"w2{k}", name=f"w2{k}")
                                 for k in range(32)]
                        for k in range(32):
                            nc.sync.dma_start(out=w2_sb[k], in_=w2_e[k])
                        for m in range(8):
                            for nch in range(BL):
                                ps = ps_f.tile([128, tsteps], F32, tag="f", name="f")
                                for k in range(32):
                                    nc.tensor.matmul(
                                        ps, w2_sb[k][:, ds(m * 128, 128)],
                                        f1_sb[k][:, ds(nch * tsteps, tsteps)],
                                        start=(k == 0), stop=(k == 31))
                                y_sb = yp.tile([128, tsteps], F32, tag="y", name="y")
                                nc.scalar.activation(
                                    out=y_sb, in_=ps, func=AF.Identity,
                                    bias=b2_t[:, ds(m, 1)], scale=1.0)
                                nc.sync.dma_start(
                                    out=yT_e[m, :, ds(nch * tsteps, tsteps)],
                                    in_=y_sb)

    _split_excess_waits(nc)
    return nc


_CACHE = {}


def _get_program(tsteps=T):
    if tsteps not in _CACHE:
        _CACHE[tsteps] = _build_program(tsteps)
    return _CACHE[tsteps]


# ----------------------------------------------------------------------------
# host-side preparation
# ----------------------------------------------------------------------------
def _bf(a):
    return np.ascontiguousarray(np.asarray(a, np.float32)).astype(BFnp)


def _prep_shared(inputs):
    d = {}
    for a in ("src", "tgt"):
        wq = np.asarray(inputs[f"{a}_wq"], np.float32)
        bq = np.asarray(inputs[f"{a}_bq"], np.float32)
        scale = HD ** -0.5
        d[f"{a}_wq"] = _bf(wq * scale)
        d[f"{a}_bq"] = np.ascontiguousarray(
            (bq * scale).reshape(8, 128).T).astype(np.float32)
        d[f"{a}_wk"] = _bf(inputs[f"{a}_wk"])
        d[f"{a}_bk"] = np.ascontiguousarray(
            np.asarray(inputs[f"{a}_bk"], np.float32).reshape(8, 128).T)
        d[f"{a}_wv"] = _bf(inputs[f"{a}_wv"])
        d[f"{a}_bv"] = _bf(inputs[f"{a}_bv"]).reshape(1, H)
        d[f"{a}_wo"] = _bf(inputs[f"{a}_wo"])
        d[f"{a}_bo"] = np.ascontiguousarray(
            np.asarray(inputs[f"{a}_bo"], np.float32).reshape(8, 128).T)

    tb = np.asarray(inputs["tgt_bias"], np.float32).reshape(T, T)  # [q, s]
    d["tgt_bT"] = np.ascontiguousarray(tb.T).astype(BFnp).reshape(4, 128, T)

    lw = np.asarray(inputs["lstm_w"], np.float32)
    lb = np.asarray(inputs["lstm_b"], np.float32)
    wx_cm = lw[:2048][:, _JIFO]
    wh_cm = lw[2048:][:, _JIFO]
    b_cm = lb[_JIFO]
    wx_pm = wx_cm.reshape(2048, 32, 128).transpose(0, 2, 1)
    b_pm = b_cm.reshape(32, 128).T
    d["Wx"] = _bf(wx_pm.reshape(2048, 4096)).reshape(16, 128, 4096)
    d["b_row"] = _bf(b_pm.reshape(1, 4096))
    d["Wh"] = _bf(wh_cm).reshape(8, 128, 4096)

    d["W1"] = _bf(inputs["ffn_w1"]).reshape(8, 128, F)
    d["b1T"] = np.ascontiguousarray(
        np.asarray(inputs["ffn_b1"], np.float32).reshape(32, 128).T)
    d["W2"] = _bf(inputs["ffn_w2"]).reshape(32, 128, H)
    d["b2T"] = np.ascontiguousarray(
        np.asarray(inputs["ffn_b2"], np.float32).reshape(8, 128).T)
    return d


def _prep_core(inputs, cid):
    b0 = cid * BL
    x = np.asarray(inputs["x"], np.float32)[b0:b0 + BL]
    mem = np.asarray(inputs["memory"], np.float32)[b0:b0 + BL]
    xT = np.ascontiguousarray(x.reshape(TOK, H).T)
    memT = np.ascontiguousarray(mem.reshape(TOK, H).T)
    sb = np.asarray(inputs["src_bias"], np.float32)[b0:b0 + BL]
    srcb = (sb.reshape(BL, 4, 128) - SHIFT).transpose(2, 0, 1).reshape(128, BL * 4)
    return {
        "xT": xT.astype(BFnp).reshape(8, 128, TOK),
        "memT": memT.astype(BFnp).reshape(8, 128, TOK),
        "src_bshift": np.ascontiguousarray(srcb).astype(np.float32),
    }


TRACE = False
LAST_RES = None


def kernel(**inputs):
    global LAST_RES
    nc = _get_program(T)
    shared = _prep_shared(inputs)
    in_maps = []
    for cid in range(NCORES):
        m = dict(shared)
        m.update(_prep_core(inputs, cid))
        in_maps.append(m)
    res = run_bass_kernel_spmd(nc, in_maps, list(range(NCORES)),
                               trace=TRACE)
    LAST_RES = res
    out = np.empty((B, T, H), np.float32)
    for cid in range(NCORES):
        yT = res.results[cid]["yT"].reshape(H, TOK)
        out[cid * BL:(cid + 1) * BL] = yT.T.reshape(BL, T, H)
    return out


if __name__ == "__main__":
    inputs = dict(np.load("/root/problem/inputs.npz"))
    got = kernel(**inputs)
    exp = np.load("/root/problem/expected64.npy")
    err = np.abs(got - exp)
    scale = np.abs(exp).max()
    print("max abs err:", err.max(), "scale-rel:", err.max() / scale)
    print("rel L2:", np.linalg.norm(got - exp) / np.linalg.norm(exp))


# revision 14
# speedup vs baseline: 133.7440x; 133.7440x over previous
"""Trainium2 Bass kernel for nn_DeepLSTMDecoderLayer.

Layer: c = srcMHA(x, memory) + tgtMHA(x, x causal); hiddens = LSTM([x, c]);
out = FFN(hiddens).   Shapes: B=16, T=S=512, H=1024, NH=16, F=4096.

Sharding: data-parallel over batch across 8 cores (B_local=2), weights
replicated.  All matmuls bf16 operands / f32 accumulate.

Layouts are feature-major ("transposed") throughout:
  - x^T / mem^T [1024 emb, 1024 tok] bf16, tok = b*512 + t
  - projections produce Q^T/K^T [emb-d, tok] and V token-major [tok, d]
  - softmax with s on partitions: exp(logits^T - 8) (constant shift cancels
    in normalization), denominator via ones-vector matmul, normalization
    applied after P@V via a rank-1 PE broadcast of 1/denom (hi+lo bf16
    split keeps it ~f32-exact)
  - LSTM: the input-part gate pre-activations G = [x,c] @ W_x + b are
    precomputed to DRAM with gate columns permuted to [j|i|f|o] and stored
    p-major (m = p*32 + c) so each step's slice is one clean [128, 32] DMA;
    the recurrence runs 512 sequential W_h-stationary steps (columns
    c-major, m = c*128 + p) accumulating into 4 per-gate-type PSUM tiles
    [128, q, b]; h-history lives in SBUF [128, q, b, slot] bf16 and feeds
    the FFN directly.
"""
import sys

if '/opt/trn_rl_repo' not in sys.path:
    sys.path.insert(0, '/opt/trn_rl_repo')

import numpy as np
import ml_dtypes
from contextlib import ExitStack

import concourse.bass as bass
import concourse.mybir as mybir
import concourse.tile as tile
import bass_rust
from concourse.bass import ds
from concourse.bass_utils import run_bass_kernel_spmd


_RUN_CACHE = {}


def _run_cached(nc, in_maps):
    """Multi-core PJRT runner with a cached jitted executable.

    Mirrors bass2jax.run_bass_via_pjrt's n_cores>1 branch, but builds the
    shard_map jit once so repeat kernel() calls skip tracing/compile.
    """
    import jax
    from jax.sharding import Mesh, PartitionSpec
    from jax.experimental.shard_map import shard_map
    from concourse import bass2jax
    from concourse.bass2jax import (_bass_exec_p, install_neuronx_cc_hook,
                                    partition_id_tensor)

    key = id(nc)
    if key not in _RUN_CACHE:
        install_neuronx_cc_hook()
        n_cores = len(in_maps)
        partition_name = (nc.partition_id_tensor.name
                          if nc.partition_id_tensor else None)
        in_names, out_names, out_avals, zero_outs = [], [], [], []
        for alloc in nc.m.functions[0].allocations:
            if not isinstance(alloc, mybir.MemoryLocationSet):
                continue
            name = alloc.memorylocations[0].name
            if alloc.kind == "ExternalInput":
                if name != partition_name:
                    in_names.append(name)
            elif alloc.kind == "ExternalOutput":
                out_names.append(name)
                shape = tuple(alloc.tensor_shape)
                dtype = mybir.dt.np(alloc.dtype)
                out_avals.append(jax.core.ShapedArray(shape, dtype))
                zero_outs.append(np.zeros(shape, dtype))
        n_params = len(in_names)
        n_outs = len(out_avals)
        in_names = in_names + out_names
        if partition_name is not None:
            in_names.append(partition_name)

        def _body(*args):
            operands = list(args)
            if partition_name is not None:
                operands.append(partition_id_tensor())
            outs = _bass_exec_p.bind(
                *operands,
                out_avals=tuple(out_avals),
                in_names=tuple(in_names),
                out_names=tuple(out_names),
                lowering_input_output_aliases=(),
                sim_require_finite=True,
                sim_require_nnan=True,
                nc=nc,
            )
            return tuple(outs)

        devices = jax.devices()[:n_cores]
        mesh = Mesh(np.asarray(devices), ("core",))
        donate = tuple(range(n_params, n_params + n_outs))
        sharded = jax.jit(
            shard_map(_body, mesh=mesh,
                      in_specs=(PartitionSpec("core"),) * (n_params + n_outs),
                      out_specs=(PartitionSpec("core"),) * n_outs,
                      check_rep=False),
            donate_argnums=donate, keep_unused=True)
        _RUN_CACHE[key] = (sharded, in_names[:n_params], out_names,
                           out_avals, zero_outs)

    c_sharded, c_ins, c_outs, c_avals, c_zeros = _RUN_CACHE[key]
    n_cores = len(in_maps)
    concat_in = [np.concatenate([np.asarray(in_maps[c][nm])
                                 for c in range(n_cores)], axis=0)
                 for nm in c_ins]
    concat_zeros = [np.zeros((n_cores * z.shape[0], *z.shape[1:]), z.dtype)
                    for z in c_zeros]
    out_arrs = c_sharded(*concat_in, *concat_zeros)
    return [{nm: np.asarray(out_arrs[i]).reshape(n_cores, *c_avals[i].shape)[c]
             for i, nm in enumerate(c_outs)}
            for c in range(n_cores)]

F32 = mybir.dt.float32
BF16 = mybir.dt.bfloat16
AF = mybir.ActivationFunctionType
BFnp = ml_dtypes.bfloat16

B, T, S, H, NH, F = 16, 512, 512, 1024, 16, 4096
HD = H // NH
NCORES = 8
BL = B // NCORES          # 2 batch items per core
TOK = BL * T              # 1024 tokens per core
SHIFT = 8.0               # constant softmax shift (cancels in normalization)

# gate-type order [j, i, f, o]: tanh block first, then contiguous sigmoids.
_JIFO = np.concatenate([np.arange(1024, 2048), np.arange(0, 1024),
                        np.arange(2048, 3072), np.arange(3072, 4096)])


# ----------------------------------------------------------------------------
# walrus workaround: this neuronx-cc rejects instructions with >1 sync wait
# ----------------------------------------------------------------------------
_wsplit_ctr = [0]


def _split_excess_waits(nc, max_waits=1):
    n = 0
    for f in nc.m.functions:
        for blk in f.blocks:
            out = []
            changed = False
            for inst in blk.instructions:
                si = inst.sync_info
                if si is not None and len(si.on_wait) > max_waits:
                    waits = list(si.on_wait)
                    excess = waits[max_waits:]
                    for i in range(0, len(excess), max_waits):
                        _wsplit_ctr[0] += 1
                        out.append(mybir.InstNoOp(
                            name=f"wsplit-{_wsplit_ctr[0]}",
                            engine=inst.engine,
                            sync_info=bass_rust.SyncInfo(
                                on_wait=excess[i:i + max_waits], on_update=[]),
                            bass_nofuse=True))
                    inst.sync_info = bass_rust.SyncInfo(
                        on_wait=waits[:max_waits], on_update=list(si.on_update))
                    changed = True
                    n += 1
                out.append(inst)
            if changed:
                blk.instructions = out
    return n


# ----------------------------------------------------------------------------
# device program
# ----------------------------------------------------------------------------
def _build_program(tsteps=T):
    nc = bass.Bass()
    dp = lambda name, shape, dt=BF16: nc.declare_dram_parameter(
        name, list(shape), dt, isOutput=False)

    xT_e = dp("xT", [8, 128, TOK])
    memT_e = dp("memT", [8, 128, TOK])
    wq_e = {a: dp(f"{a}_wq", [H, H]) for a in ("src", "tgt")}
    wk_e = {a: dp(f"{a}_wk", [H, H]) for a in ("src", "tgt")}
    wv_e = {a: dp(f"{a}_wv", [H, H]) for a in ("src", "tgt")}
    wo_e = {a: dp(f"{a}_wo", [H, H]) for a in ("src", "tgt")}
    bq_e = {a: dp(f"{a}_bq", [128, 8], F32) for a in ("src", "tgt")}
    bk_e = {a: dp(f"{a}_bk", [128, 8], F32) for a in ("src", "tgt")}
    bv_e = {a: dp(f"{a}_bv", [1, H]) for a in ("src", "tgt")}
    bo_e = {a: dp(f"{a}_bo", [128, 8], F32) for a in ("src", "tgt")}
    srcb_e = dp("src_bshift", [128, BL * 4], F32)     # src_bias - SHIFT
    tgtb_e = dp("tgt_bT", [4, 128, T])                # bias^T [s, q] bf16
    wx_e = dp("Wx", [16, 128, 4096])
    brow_e = dp("b_row", [1, 4096])
    wh_e = dp("Wh", [8, 128, 4096])
    w1_e = dp("W1", [8, 128, F])
    b1_e = dp("b1T", [128, 32], F32)
    w2_e = dp("W2", [32, 128, H])
    b2_e = dp("b2T", [128, 8], F32)

    otok = BL * tsteps
    yT_e = nc.declare_dram_parameter("yT", [8, 128, otok], F32, isOutput=True)
    G_d = nc.dram_tensor("Gd", [TOK, 128, 32], F32)

    with tile.TileContext(nc) as tc:
        with ExitStack() as top:
            persist = top.enter_context(tc.tile_pool(name="persist", bufs=1))
            smalls = top.enter_context(tc.tile_pool(name="smalls", bufs=1))

            xT = [persist.tile([128, TOK], BF16, tag=f"xT{k}", name=f"xT{k}") for k in range(8)]
            cT = [[persist.tile([128, T], BF16, tag=f"cT{b}_{m}", name=f"cT{b}_{m}")
                   for m in range(8)] for b in range(BL)]
            for k in range(8):
                nc.sync.dma_start(out=xT[k], in_=xT_e[k])

            ones_col = smalls.tile([1, 128], BF16, tag="ones_col", name="ones_col")
            nc.vector.memset(ones_col, 1.0)
            ones_128 = smalls.tile([128, 1], BF16, tag="ones_128", name="ones_128")
            nc.vector.memset(ones_128, 1.0)
            ones_1x64 = smalls.tile([1, 64], BF16, tag="ones_1x64", name="ones_1x64")
            nc.vector.memset(ones_1x64, 1.0)
            srcb_t = smalls.tile([128, BL * 4], F32, tag="srcb", name="srcb")
            nc.sync.dma_start(out=srcb_t, in_=srcb_e[:, :])
            negshift = smalls.tile([128, 1], F32, tag="negshift", name="negshift")
            nc.vector.memset(negshift, -SHIFT)

            # ---------------- phase A: both attentions -> cT ----------------
            with tc.tile_pool(name="attc", bufs=1) as attc, \
                 tc.tile_pool(name="wstream", bufs=2) as wpool, \
                 tc.tile_pool(name="qkv", bufs=1) as qkv, \
                 tc.tile_pool(name="epool", bufs=2) as epool, \
                 tc.tile_pool(name="ps_lt", bufs=4, space="PSUM") as ps_lt, \
                 tc.tile_pool(name="ps_s", bufs=1, space="PSUM") as ps_s, \
                 tc.tile_pool(name="ps_o", bufs=1, space="PSUM") as ps_o, \
                 tc.tile_pool(name="ps_rb", bufs=1, space="PSUM") as ps_rb:

                memT = [attc.tile([128, TOK], BF16, tag=f"memT{k}", name=f"memT{k}")
                        for k in range(8)]
                for k in range(8):
                    nc.sync.dma_start(out=memT[k], in_=memT_e[k])
                tgtb = [attc.tile([128, T], BF16, tag=f"tgtb{st}", name=f"tgtb{st}")
                        for st in range(4)]
                for st in range(4):
                    nc.sync.dma_start(out=tgtb[st], in_=tgtb_e[st])

                for attn in ("src", "tgt"):
                    kvT = memT if attn == "src" else xT

                    def load_w(ext):
                        w_sb = [wpool.tile([128, H], BF16, tag=f"w{k}", name=f"w{k}")
                                for k in range(8)]
                        for k in range(8):
                            nc.sync.dma_start(out=w_sb[k],
                                              in_=ext[ds(k * 128, 128), :])
                        return w_sb

                    bq_t = smalls.tile([128, 8], F32, tag=f"bq_{attn}", name=f"bq_{attn}")
                    nc.sync.dma_start(out=bq_t, in_=bq_e[attn][:, :])
                    bk_t = smalls.tile([128, 8], F32, tag=f"bk_{attn}", name=f"bk_{attn}")
                    nc.sync.dma_start(out=bk_t, in_=bk_e[attn][:, :])
                    bv_t = smalls.tile([1, H], BF16, tag=f"bv_{attn}", name=f"bv_{attn}")
                    nc.sync.dma_start(out=bv_t, in_=bv_e[attn][:, :])

                    # --- projections: Q^T, K^T (emb-major), V (token-major)
                    QT, KT, VV = [], [], []
                    w_sb = load_w(wq_e[attn])
                    for m in range(8):
                        q_sb = qkv.tile([128, TOK], BF16, tag=f"qT{m}", name=f"qT{m}")
                        QT.append(q_sb)
                        for nch in range(2):
                            ps = ps_lt.tile([128, 512], F32, tag="lt", name="lt")
                            for k in range(8):
                                nc.tensor.matmul(
                                    ps, w_sb[k][:, ds(m * 128, 128)],
                                    xT[k][:, ds(nch * 512, 512)],
                                    start=(k == 0), stop=(k == 7))
                            nc.scalar.activation(
                                out=q_sb[:, ds(nch * 512, 512)], in_=ps,
                                func=AF.Identity, bias=bq_t[:, ds(m, 1)], scale=1.0)
                    w_sb = load_w(wk_e[attn])
                    for m in range(8):
                        k_sb = qkv.tile([128, TOK], BF16, tag=f"kT{m}", name=f"kT{m}")
                        KT.append(k_sb)
                        for nch in range(2):
                            ps = ps_lt.tile([128, 512], F32, tag="lt", name="lt")
                            for k in range(8):
                                nc.tensor.matmul(
                                    ps, w_sb[k][:, ds(m * 128, 128)],
                                    kvT[k][:, ds(nch * 512, 512)],
                                    start=(k == 0), stop=(k == 7))
                            nc.scalar.activation(
                                out=k_sb[:, ds(nch * 512, 512)], in_=ps,
                                func=AF.Identity, bias=bk_t[:, ds(m, 1)], scale=1.0)
                    w_sb = load_w(wv_e[attn])
                    for st in range(8):   # global token tile (b*4 + s_tile)
                        v_sb = qkv.tile([128, H], BF16, tag=f"v{st}", name=f"v{st}")
                        VV.append(v_sb)
                        for nch in range(2):
                            ps = ps_lt.tile([128, 512], F32, tag="lt", name="lt")
                            for k in range(8):
                                nc.tensor.matmul(
                                    ps, kvT[k][:, ds(st * 128, 128)],
                                    w_sb[k][:, ds(nch * 512, 512)],
                                    start=(k == 0), stop=False)
                            nc.tensor.matmul(
                                ps, ones_col, bv_t[:, ds(nch * 512, 512)],
                                start=False, stop=True)
                            nc.vector.tensor_copy(
                                out=v_sb[:, ds(nch * 512, 512)], in_=ps)

                    # --- attention core, concat heads into oT ---
                    oT = [[qkv.tile([128, T], BF16, tag=f"oT{b}_{mm}", name=f"oT{b}_{mm}")
                           for mm in range(8)] for b in range(BL)]
                    for b in range(BL):
                        for h in range(NH):
                            mh, off = h // 2, 64 * (h % 2)
                            lts = []
                            for st in range(4):
                                ps = ps_lt.tile([128, 512], F32, tag="lt", name="lt")
                                lts.append(ps)
                                nc.tensor.matmul(
                                    ps,
                                    KT[mh][ds(off, 64),
                                           ds(b * 512 + st * 128, 128)],
                                    QT[mh][ds(off, 64), ds(b * 512, 512)],
                                    start=True, stop=True)
                            es = []
                            for st in range(4):
                                e_sb = epool.tile([128, 512], BF16,
                                                  tag=f"e{st}", name=f"e{st}")
                                es.append(e_sb)
                                if attn == "tgt":
                                    nc.vector.tensor_add(lts[st], lts[st],
                                                         tgtb[st])
                                    nc.scalar.activation(
                                        out=e_sb, in_=lts[st], func=AF.Exp,
                                        bias=negshift[:, :], scale=1.0)
                                else:
                                    nc.scalar.activation(
                                        out=e_sb, in_=lts[st], func=AF.Exp,
                                        bias=srcb_t[:, ds(b * 4 + st, 1)], scale=1.0)
                            s_ps = ps_s.tile([1, 512], F32, tag="s", name="s")
                            for st in range(4):
                                nc.tensor.matmul(s_ps, ones_128, es[st],
                                                 start=(st == 0),
                                                 stop=(st == 3))
                            r_sb = epool.tile([1, 512], F32, tag="r", name="r")
                            nc.vector.reciprocal(out=r_sb, in_=s_ps)
                            # hi+lo bf16 split of 1/denom (keeps ~f32 accuracy
                            # through the bf16 rank-1 broadcast): two
                            # accumulating K=1 matmuls
                            r_hi = epool.tile([1, 512], BF16, tag="rhi", name="rhi")
                            nc.vector.tensor_copy(out=r_hi, in_=r_sb)
                            r_lo = epool.tile([1, 512], BF16, tag="rlo", name="rlo")
                            nc.vector.tensor_sub(r_lo, r_sb, r_hi)
                            o_ps = ps_o.tile([64, 512], F32, tag="o", name="o")
                            for st in range(4):
                                nc.tensor.matmul(
                                    o_ps,
                                    VV[b * 4 + st][:, ds(h * 64, 64)],
                                    es[st], start=(st == 0), stop=(st == 3))
                            rb_ps = ps_rb.tile([64, 512], F32, tag="rb", name="rb")
                            nc.tensor.matmul(rb_ps, ones_1x64, r_hi,
                                             start=True, stop=False)
                            nc.tensor.matmul(rb_ps, ones_1x64, r_lo,
                                             start=False, stop=True)
                            o_f = epool.tile([64, 512], F32, tag="of", name="of")
                            nc.vector.tensor_copy(out=o_f, in_=o_ps)
                            nc.vector.tensor_mul(
                                oT[b][mh][ds(off, 64), :], o_f, rb_ps)

                    # --- out-projection into cT (src writes, tgt adds) ---
                    bo_t = smalls.tile([128, 8], F32, tag=f"bo_{attn}", name=f"bo_{attn}")
                    nc.sync.dma_start(out=bo_t, in_=bo_e[attn][:, :])
                    w_sb = load_w(wo_e[attn])
                    for b in range(BL):
                        for m in range(8):
                            ps = ps_lt.tile([128, 512], F32, tag="lt", name="lt")
                            for k in range(8):
                                nc.tensor.matmul(
                                    ps, w_sb[k][:, ds(m * 128, 128)],
                                    oT[b][k], start=(k == 0), stop=(k == 7))
                            if attn == "src":
                                nc.scalar.activation(
                                    out=cT[b][m], in_=ps, func=AF.Identity,
                                    bias=bo_t[:, ds(m, 1)], scale=1.0)
                            else:
                                tmp = epool.tile([128, 512], F32, tag="ctmp", name="ctmp")
                                nc.scalar.activation(
                                    out=tmp, in_=ps, func=AF.Identity,
                                    bias=bo_t[:, ds(m, 1)], scale=1.0)
                                nc.vector.tensor_add(cT[b][m], cT[b][m], tmp)

            # ---------------- phase B: G = [x, c] @ Wx + b -> DRAM ----------
            with tc.tile_pool(name="wx", bufs=1) as wxp, \
                 tc.tile_pool(name="gstage", bufs=4) as gst, \
                 tc.tile_pool(name="ps_g", bufs=4, space="PSUM") as ps_g:
                wx_sb = [wxp.tile([128, 4096], BF16, tag=f"wx{k}", name=f"wx{k}")
                         for k in range(16)]
                for k in range(16):
                    nc.sync.dma_start(out=wx_sb[k], in_=wx_e[k])
                brow = smalls.tile([1, 4096], BF16, tag="brow", name="brow")
                nc.sync.dma_start(out=brow, in_=brow_e[:, :])

                for b in range(BL):
                    for tt in range(4):
                        for nch in range(8):
                            ps = ps_g.tile([128, 512], F32, tag="g", name="g")
                            for k in range(8):
                                nc.tensor.matmul(
                                    ps, xT[k][:, ds(b * 512 + tt * 128, 128)],
                                    wx_sb[k][:, ds(nch * 512, 512)],
                                    start=(k == 0), stop=False)
                            for k in range(8):
                                nc.tensor.matmul(
                                    ps, cT[b][k][:, ds(tt * 128, 128)],
                                    wx_sb[8 + k][:, ds(nch * 512, 512)],
                                    start=False, stop=False)
                            nc.tensor.matmul(
                                ps, ones_col, brow[:, ds(nch * 512, 512)],
                                start=False, stop=True)
                            g_sb = gst.tile([128, 512], F32, tag="gsb", name="gsb")
                            nc.vector.tensor_copy(out=g_sb, in_=ps)
                            r0 = b * 512 + tt * 128
                            nc.sync.dma_start(
                                out=G_d[ds(r0, 128), ds(nch * 16, 16), :],
                                in_=g_sb.rearrange("p (a c) -> p a c", a=16))

            # ---------------- phases C+D: recurrence then FFN ---------------
            with tc.tile_pool(name="hstate", bufs=1) as hsp:
                hT = hsp.tile([128, 8, BL, tsteps + 1], BF16, tag="hT", name="hT")
                nc.vector.memset(hT[:, :, :, 0], 0.0)

                with tc.tile_pool(name="wh", bufs=1) as whp, \
                     tc.tile_pool(name="gstep", bufs=1) as gsp, \
                     tc.tile_pool(name="gw", bufs=2) as gwp, \
                     tc.tile_pool(name="ps_r", bufs=2, space="PSUM") as ps_r:
                    wh_sb = [whp.tile([128, 4096], BF16, tag=f"wh{k}", name=f"wh{k}")
                             for k in range(8)]
                    for k in range(8):
                        nc.sync.dma_start(out=wh_sb[k], in_=wh_e[k])
                    c_st = hsp.tile([128, 8, BL], F32, tag="c_st", name="c_st")
                    nc.vector.memset(c_st, 0.0)
                    hstep = hsp.tile([128, 8, BL], BF16, tag="hstep", name="hstep")
                    nc.vector.memset(hstep, 0.0)
                    g_t = gsp.tile([128, 32, BL], F32, tag="g_t", name="g_t")

                    with tc.For_i(0, tsteps, 1) as t:
                        for b in range(BL):
                            nc.sync.dma_start(
                                out=g_t[:, :, b],
                                in_=G_d[ds(t + b * 512, 1), :, :].rearrange(
                                    "one p c -> (one p) c"))
                        psg = [ps_r.tile([128, 8, BL], F32, tag=f"ps{tau}", name=f"ps{tau}")
                               for tau in range(4)]
                        ga = [gwp.tile([128, 8, BL], F32, tag=f"ga{tau}", name=f"ga{tau}")
                              for tau in range(4)]
                        tj = gwp.tile([128, 8, BL], F32, tag="tj", name="tj")
                        si = gwp.tile([128, 8, BL], F32, tag="si", name="si")
                        sf = gwp.tile([128, 8, BL], F32, tag="sf", name="sf")
                        so = gwp.tile([128, 8, BL], F32, tag="so", name="so")
                        t1 = gwp.tile([128, 8, BL], F32, tag="t1", name="t1")
                        m1 = gwp.tile([128, 8, BL], F32, tag="m1", name="m1")
                        tcn = gwp.tile([128, 8, BL], F32, tag="tcn", name="tcn")

                        for tau in range(4):
                            for q in range(8):
                                c = tau * 8 + q
                                for k in range(8):
                                    nc.tensor.matmul(
                                        psg[tau][:, q, :],
                                        wh_sb[k][:, ds(c * 128, 128)],
                                        hstep[:, k, :],
                                        start=(k == 0), stop=(k == 7))
                            nc.vector.tensor_add(
                                ga[tau], psg[tau], g_t[:, ds(tau * 8, 8), :])
                            if tau == 0:
                                nc.scalar.activation(out=tj, in_=ga[0],
                                                     func=AF.Tanh)
                            elif tau == 1:
                                nc.scalar.activation(out=si, in_=ga[1],
                                                     func=AF.Sigmoid)
                                nc.vector.tensor_mul(t1, si, tj)
                            elif tau == 2:
                                nc.scalar.activation(out=sf, in_=ga[2],
                                                     func=AF.Sigmoid)
                                nc.vector.tensor_mul(m1, sf, c_st)
                                nc.vector.tensor_add(c_st, t1, m1)
                                nc.scalar.activation(out=tcn, in_=c_st,
                                                     func=AF.Tanh)
                            else:
                                nc.scalar.activation(out=so, in_=ga[3],
                                                     func=AF.Sigmoid)
                                nc.vector.tensor_mul(hstep, so, tcn)
                                nc.vector.tensor_mul(
                                    hT[:, :, :, ds(t + 1, 1)], so, tcn)

                # ---------------- phase D: FFN --------------------------
                b1_t = smalls.tile([128, 32], F32, tag="b1t", name="b1t")
                nc.sync.dma_start(out=b1_t, in_=b1_e[:, :])
                b2_t = smalls.tile([128, 8], F32, tag="b2t", name="b2t")
                nc.sync.dma_start(out=b2_t, in_=b2_e[:, :])

                with tc.tile_pool(name="f1", bufs=1) as f1p, \
                     tc.tile_pool(name="ps_f", bufs=4, space="PSUM") as ps_f:
                    f1_sb = [f1p.tile([128, BL * tsteps], BF16, tag=f"f1{fc}", name=f"f1{fc}")
                             for fc in range(32)]
                    with tc.tile_pool(name="w1", bufs=1) as w1p:
                        w1_sb = [w1p.tile([128, F], BF16, tag=f"w1{k}", name=f"w1{k}")
                                 for k in range(8)]
                        for k in range(8):
                            nc.sync.dma_start(out=w1_sb[k], in_=w1_e[k])
                        for fc in range(32):
                            for b in range(BL):
                                ps = ps_f.tile([128, tsteps], F32, tag="f", name="f")
                                for k in range(8):
                                    nc.tensor.matmul(
                                        ps, w1_sb[k][:, ds(fc * 128, 128)],
                                        hT[:, k, b, 1:tsteps + 1],
                                        start=(k == 0), stop=(k == 7))
                                nc.scalar.activation(
                                    out=f1_sb[fc][:, ds(b * tsteps, tsteps)],
                                    in_=ps, func=AF.Relu,
                                    bias=b1_t[:, ds(fc, 1)], scale=1.0)

                    with tc.tile_pool(name="w2", bufs=1) as w2p, \
                         tc.tile_pool(name="yout", bufs=4) as yp:
                        w2_sb = [w2p.tile([128, H], BF16, tag=f"w2{k}", name=f"w2{k}")
                                 for k in range(32)]
                        for k in range(32):
                            nc.sync.dma_start(out=w2_sb[k], in_=w2_e[k])
                        for m in range(8):
                            for nch in range(BL):
                                ps = ps_f.tile([128, tsteps], F32, tag="f", name="f")
                                for k in range(32):
                                    nc.tensor.matmul(
                                        ps, w2_sb[k][:, ds(m * 128, 128)],
                                        f1_sb[k][:, ds(nch * tsteps, tsteps)],
                                        start=(k == 0), stop=(k == 31))
                                y_sb = yp.tile([128, tsteps], F32, tag="y", name="y")
                                nc.scalar.activation(
                                    out=y_sb, in_=ps, func=AF.Identity,
                                    bias=b2_t[:, ds(m, 1)], scale=1.0)
                                nc.sync.dma_start(
                                    out=yT_e[m, :, ds(nch * tsteps, tsteps)],
                                    in_=y_sb)

    _split_excess_waits(nc)
    return nc


_CACHE = {}


def _get_program(tsteps=T):
    if tsteps not in _CACHE:
        _CACHE[tsteps] = _build_program(tsteps)
    return _CACHE[tsteps]


# ----------------------------------------------------------------------------
# host-side preparation
# ----------------------------------------------------------------------------
def _bf(a):
    return np.ascontiguousarray(np.asarray(a, np.float32)).astype(BFnp)


def _prep_shared(inputs):
    d = {}
    for a in ("src", "tgt"):
        wq = np.asarray(inputs[f"{a}_wq"], np.float32)
        bq = np.asarray(inputs[f"{a}_bq"], np.float32)
        scale = HD ** -0.5
        d[f"{a}_wq"] = _bf(wq * scale)
        d[f"{a}_bq"] = np.ascontiguousarray(
            (bq * scale).reshape(8, 128).T).astype(np.float32)
        d[f"{a}_wk"] = _bf(inputs[f"{a}_wk"])
        d[f"{a}_bk"] = np.ascontiguousarray(
            np.asarray(inputs[f"{a}_bk"], np.float32).reshape(8, 128).T)
        d[f"{a}_wv"] = _bf(inputs[f"{a}_wv"])
        d[f"{a}_bv"] = _bf(inputs[f"{a}_bv"]).reshape(1, H)
        d[f"{a}_wo"] = _bf(inputs[f"{a}_wo"])
        d[f"{a}_bo"] = np.ascontiguousarray(
            np.asarray(inputs[f"{a}_bo"], np.float32).reshape(8, 128).T)

    tb = np.asarray(inputs["tgt_bias"], np.float32).reshape(T, T)  # [q, s]
    d["tgt_bT"] = np.ascontiguousarray(tb.T).astype(BFnp).reshape(4, 128, T)

    lw = np.asarray(inputs["lstm_w"], np.float32)
    lb = np.asarray(inputs["lstm_b"], np.float32)
    wx_cm = lw[:2048][:, _JIFO]
    wh_cm = lw[2048:][:, _JIFO]
    b_cm = lb[_JIFO]
    wx_pm = wx_cm.reshape(2048, 32, 128).transpose(0, 2, 1)
    b_pm = b_cm.reshape(32, 128).T
    d["Wx"] = _bf(wx_pm.reshape(2048, 4096)).reshape(16, 128, 4096)
    d["b_row"] = _bf(b_pm.reshape(1, 4096))
    d["Wh"] = _bf(wh_cm).reshape(8, 128, 4096)

    d["W1"] = _bf(inputs["ffn_w1"]).reshape(8, 128, F)
    d["b1T"] = np.ascontiguousarray(
        np.asarray(inputs["ffn_b1"], np.float32).reshape(32, 128).T)
    d["W2"] = _bf(inputs["ffn_w2"]).reshape(32, 128, H)
    d["b2T"] = np.ascontiguousarray(
        np.asarray(inputs["ffn_b2"], np.float32).reshape(8, 128).T)
    return d


def _prep_core(inputs, cid):
    b0 = cid * BL
    x = np.asarray(inputs["x"], np.float32)[b0:b0 + BL]
    mem = np.asarray(inputs["memory"], np.float32)[b0:b0 + BL]
    xT = np.ascontiguousarray(x.reshape(TOK, H).T)
    memT = np.ascontiguousarray(mem.reshape(TOK, H).T)
    sb = np.asarray(inputs["src_bias"], np.float32)[b0:b0 + BL]
    srcb = (sb.reshape(BL, 4, 128) - SHIFT).transpose(2, 0, 1).reshape(128, BL * 4)
    return {
        "xT": xT.astype(BFnp).reshape(8, 128, TOK),
        "memT": memT.astype(BFnp).reshape(8, 128, TOK),
        "src_bshift": np.ascontiguousarray(srcb).astype(np.float32),
    }


TRACE = False
LAST_RES = None


def kernel(**inputs):
    global LAST_RES
    nc = _get_program(T)
    shared = _prep_shared(inputs)
    in_maps = []
    for cid in range(NCORES):
        m = dict(shared)
        m.update(_prep_core(inputs, cid))
        in_maps.append(m)
    results = _run_cached(nc, in_maps)
    LAST_RES = results
    out = np.empty((B, T, H), np.float32)
    for cid in range(NCORES):
        yT = results[cid]["yT"].reshape(H, TOK)
        out[cid * BL:(cid + 1) * BL] = yT.T.reshape(BL, T, H)
    return out


if __name__ == "__main__":
    inputs = dict(np.load("/root/problem/inputs.npz"))
    got = kernel(**inputs)
    exp = np.load("/root/problem/expected64.npy")
    err = np.abs(got - exp)
    scale = np.abs(exp).max()
    print("max abs err:", err.max(), "scale-rel:", err.max() / scale)
    print("rel L2:", np.linalg.norm(got - exp) / np.linalg.norm(exp))
